# revision 53
# baseline (speedup 1.0000x reference)
"""Trainium2 Bass kernel for a segmented tensor-product contraction.

Computation (per batch row z, channel u, segments of width U=128):
  out[z, so, u] += c_p * x0[i0[z], s0_p, u] * prod_k x1[z, sk_p, u]
for 256 paths of degree 1..3 over S=16 segments.

Strategy (v2):
  - Data-parallel over z across 8 NeuronCores (512 rows each).
  - One big SBUF "arena" of [128 x 512z] bf16 slots: x1t segs | x0g segs |
    shared aux | phase-overlaid aux | squares | pins | term pool | out
    staging.  Every DVE op is a strided multi-slot TENSOR_TENSOR over
    arena slots; any two tiles merge (arbitrary slot strides via AP
    middle-dim striding), longer runs merge on arithmetic progressions.
    This amortizes the ~150 ns per-instruction DVE overhead measured on
    hardware against the ~267 ns/tile streaming cost.
  - x0 row gather via one-hot matmuls on PE (doubles as PE warmup).
  - Joint CSE over all 256 paths into shared sg(s0,s)/pair(a,b) products;
    squares x1[s]^2 go to the otherwise-idle ACT engine.
  - Coefficient scaling + accumulation on PE: diag(c) matmul per path use
    into per-so PSUM banks (2 phases x 8 banks), exact f32 adds.
  - PE keep-warm: zero-diag filler matmuls during DVE-only stretches keep
    the HAM clock gate at 2.4 GHz.
"""

import os
from collections import Counter, defaultdict

import numpy as np

U = 128
S = 16
NELEM = 64
Z = 4096
NCORES = 8
ZS = Z // NCORES  # 512 rows per core

LAST_EXEC_NS = None
LAST_RESULTS = None

F32 = "float32"
SLAB = 16  # coefficient-diagonal matrices per DMA slab
MAX_ARENA = 196  # max arena slots (KB/partition budget)


# --------------------------------------------------------------------------
# planning
# --------------------------------------------------------------------------

def _parse_paths(idxs, coeffs):
    paths = []  # (degree, x1segs_sorted, s0, so, coeff)
    for idx, cf in zip(idxs, coeffs):
        d = idx.shape[1] - 2
        for r, c in zip(idx, cf):
            r = [int(v) for v in r]
            paths.append((d, tuple(sorted(r[:d])), r[d], r[d + 1], float(c)))
    return paths


def _pairkey(a, b):
    if a == b:
        return ("sq", a)
    return ("pair", (min(a, b), max(a, b)))


def _mono_options(segs, s0):
    """Split options for a distinct d>=2 monomial: (aux_keys, (refA, refB)).
    refs: ('x1',s) ('x0g',s0) ('sg',(s0,s)) ('pair',(a,b)) ('sq',a)."""
    d = len(segs)
    if d == 2:
        a, b = segs
        if a == b:
            return [
                ((("sq", a),), (("sq", a), ("x0g", s0))),
                ((("sg", (s0, a)),), (("sg", (s0, a)), ("x1", a))),
            ]
        return [
            ((("sg", (s0, a)),), (("sg", (s0, a)), ("x1", b))),
            ((("sg", (s0, b)),), (("sg", (s0, b)), ("x1", a))),
            ((("pair", (a, b)),), (("pair", (a, b)), ("x0g", s0))),
        ]
    a, b, c = segs
    opts = []
    seen = set()
    for x, (y, z) in ((a, (b, c)), (b, (a, c)), (c, (a, b))):
        pk = _pairkey(y, z)
        key = (("sg", (s0, x)), pk)
        if key in seen:
            continue
        seen.add(key)
        opts.append((key, (pk, ("sg", (s0, x)))))
    return opts


def _choose_splits(monos, d1sgs, n_sweeps=8, seed=0):
    import random

    rng = random.Random(seed)
    keys = [m for m in monos if len(m[0]) >= 2]
    opts = {m: _mono_options(*m) for m in keys}
    choice = {m: 0 for m in keys}
    counts = defaultdict(int)

    def kcost(k):
        if k[0] == "sq":
            return 0.02
        if k in d1sgs:
            return 0.0
        return 1.0

    for m in keys:
        for k in opts[m][choice[m]][0]:
            counts[k] += 1
    for _ in range(n_sweeps):
        changed = False
        order = keys[:]
        rng.shuffle(order)
        for m in order:
            cur = choice[m]
            best, best_c = cur, None
            for ci, (aux, _) in enumerate(opts[m]):
                cost = 0.0
                for k in aux:
                    others = counts[k] - (1 if k in opts[m][cur][0] else 0)
                    cost += kcost(k) / (1 + others)
                if best_c is None or cost < best_c - 1e-9:
                    best, best_c = ci, cost
            if best != cur:
                for k in opts[m][cur][0]:
                    counts[k] -= 1
                for k in opts[m][best][0]:
                    counts[k] += 1
                choice[m] = best
                changed = True
        if not changed:
            break
    forms = {m: opts[m][choice[m]][1] for m in keys}
    aux = set()
    for m in keys:
        for k in opts[m][choice[m]][0]:
            aux.add(k)
    return forms, aux


MAXD = 63  # max |slot stride| — TENSOR3D step_elem is 16-bit (x512 elems)


def _merge_ops(items):
    """Group (A_slot, B_slot) ops into strided runs.

    Returns list of (member_idx_list, a0, da, b0, db); strides in slots,
    0 = broadcast, |stride| <= MAXD.  Members are ordered; dests assigned
    contiguously by the caller in member order."""
    remaining = set(range(len(items)))
    runs = []
    banned = set()
    while True:
        cnt = Counter()
        for i in remaining:
            a, b = items[i]
            cnt[a] += 1
            cnt[b] += 1
        cand = [(n, v) for v, n in cnt.items() if n >= 2 and v not in banned]
        if not cand:
            break
        _, v = max(cand)
        grp = [i for i in remaining if v in items[i]]
        if len(grp) < 2:
            banned.add(v)
            continue

        def other(i):
            a, b = items[i]
            return b if a == v else a

        others = sorted(grp, key=other)
        chains = []
        cur = [others[0]]
        for i in others[1:]:
            d = other(i) - other(cur[-1])
            if len(cur) == 1:
                if abs(d) <= MAXD:
                    cur.append(i)
                else:
                    chains.append(cur)
                    cur = [i]
            elif d == other(cur[1]) - other(cur[0]):
                cur.append(i)
            else:
                chains.append(cur)
                cur = [i]
        chains.append(cur)
        got = False
        for ch in chains:
            if len(ch) >= 2:
                got = True
                da = other(ch[1]) - other(ch[0])
                runs.append((ch, other(ch[0]), da, v, 0))
                for i in ch:
                    remaining.discard(i)
        if not got:
            banned.add(v)

    def pair_run(i1, i2):
        (a1, b1), (a2, b2) = items[i1], items[i2]
        if abs(a2 - a1) <= MAXD and abs(b2 - b1) <= MAXD:
            return ([i1, i2], a1, a2 - a1, b1, b2 - b1)
        if abs(b2 - a1) <= MAXD and abs(a2 - b1) <= MAXD:
            return ([i1, i2], a1, b2 - a1, b1, a2 - b1)
        return None

    left = sorted(remaining, key=lambda i: items[i])
    used = set()
    for idx, i1 in enumerate(left):
        if i1 in used:
            continue
        for i2 in left[idx + 1:]:
            if i2 in used:
                continue
            r = pair_run(i1, i2)
            if r is not None:
                runs.append(r)
                used.add(i1)
                used.add(i2)
                break
        else:
            runs.append(([i1], items[i1][0], 1, items[i1][1], 1))
            used.add(i1)
    return runs


def _merge_builds(triples):
    """Merge build ops with FIXED dest slots: (dest, a, b) ->
    runs (d0, dd, k, a0, da, b0, db), all |strides| <= MAXD."""
    remaining = set(range(len(triples)))
    runs = []
    banned = set()
    while True:
        cnt = Counter()
        for i in remaining:
            _, a, b = triples[i]
            cnt[a] += 1
            cnt[b] += 1
        cand = [(n, v) for v, n in cnt.items() if n >= 2 and v not in banned]
        if not cand:
            break
        _, v = max(cand)
        grp = [i for i in remaining if v in triples[i][1:3]]
        if len(grp) < 2:
            banned.add(v)
            continue

        def other(i):
            _, a, b = triples[i]
            return b if a == v else a

        members = sorted(grp, key=lambda i: triples[i][0])
        chains = []
        cur = [members[0]]
        for i in members[1:]:
            dd = triples[i][0] - triples[cur[-1]][0]
            do = other(i) - other(cur[-1])
            if len(cur) == 1:
                if abs(dd) <= MAXD and abs(do) <= MAXD:
                    cur.append(i)
                else:
                    chains.append(cur)
                    cur = [i]
            elif (dd == triples[cur[1]][0] - triples[cur[0]][0]
                  and do == other(cur[1]) - other(cur[0])):
                cur.append(i)
            else:
                chains.append(cur)
                cur = [i]
        chains.append(cur)
        got = False
        for ch in chains:
            if len(ch) >= 2:
                got = True
                dd = triples[ch[1]][0] - triples[ch[0]][0]
                do = other(ch[1]) - other(ch[0])
                runs.append((triples[ch[0]][0], dd, len(ch),
                             other(ch[0]), do, v, 0))
                remaining -= set(ch)
        if not got:
            banned.add(v)
    left = sorted(remaining, key=lambda i: triples[i])
    used = set()
    for idx, i1 in enumerate(left):
        if i1 in used:
            continue
        got = None
        for i2 in left[idx + 1:]:
            if i2 in used:
                continue
            d1, a1, b1 = triples[i1]
            d2, a2, b2 = triples[i2]
            if abs(d2 - d1) > MAXD:
                continue
            if abs(a2 - a1) <= MAXD and abs(b2 - b1) <= MAXD:
                got = (i2, (d1, d2 - d1, 2, a1, a2 - a1, b1, b2 - b1))
                break
            if abs(b2 - a1) <= MAXD and abs(a2 - b1) <= MAXD:
                got = (i2, (d1, d2 - d1, 2, a1, b2 - a1, b1, a2 - b1))
                break
        if got:
            i2, run = got
            runs.append(run)
            used.add(i1)
            used.add(i2)
        else:
            d1, a1, b1 = triples[i1]
            runs.append((d1, 1, 1, a1, 1, b1, 1))
            used.add(i1)
    return runs


def _plan(idxs, coeffs, npool):
    paths = _parse_paths(idxs, coeffs)
    monos = defaultdict(list)  # (segs, s0) -> [(so, c)]
    for d, segs, s0, so, c in paths:
        monos[(segs, s0)].append((so, c))
    monos = dict(monos)
    d1sgs = set(("sg", (m[1], m[0][0])) for m in monos if len(m[0]) == 1)
    forms, aux = _choose_splits(monos, d1sgs)
    for k in d1sgs:
        aux.add(k)

    def phase_of_so(so):
        return 0 if so < 8 else 1

    # phase usage of each aux key and monomial
    aux_phases = defaultdict(set)
    mono_phase = {}
    for m, uses in monos.items():
        ph = set(phase_of_so(so) for so, _ in uses)
        mono_phase[m] = ph
        if len(m[0]) == 1:
            for p in ph:
                aux_phases[("sg", (m[1], m[0][0]))].add(p)
        else:
            for r in forms[m]:
                if r[0] in ("sg", "pair", "sq"):
                    for p in ph:
                        aux_phases[r].add(p)

    sqs = sorted(k for k in aux if k[0] == "sq")
    shared = sorted(k for k in aux if k[0] != "sq" and len(aux_phases[k]) != 1)
    only = {ph: sorted(k for k in aux
                       if k[0] != "sq" and aux_phases[k] == {ph})
            for ph in (0, 1)}

    # ---- term refs per phase ----------------------------------------
    termref = {0: [], 1: []}  # (mono, refA, refB)
    for m in sorted(monos):
        if len(m[0]) < 2:
            continue
        ph = min(mono_phase[m])
        termref[ph].append((m, forms[m][0], forms[m][1]))

    def region_of(k):
        if len(aux_phases[k]) != 1:
            return "S"
        return "A" if aux_phases[k] == {0} else "B"

    # ---- star grouping: pick hubs shared by >=3 terms; their varying
    # aux operands get CONSECUTIVE slots so one TT builds the whole star.
    place = {"S": [], "A": [], "B": []}  # aux keys in placement order
    placed = set()
    star_runs = {0: [], 1: []}  # (member_idx_list, region, other_base_idx, hub)
    leftover = {0: [], 1: []}
    for ph in (0, 1):
        terms_ph = termref[ph]
        unused = set(range(len(terms_ph)))
        banned = set()
        while True:
            cnt = Counter()
            for i in unused:
                _, ra, rb = terms_ph[i]
                cnt[ra] += 1
                cnt[rb] += 1
            best = None
            for h, n in cnt.most_common():
                if n < 3:
                    break
                if h in banned:
                    continue
                mems = [i for i in unused if h in terms_ph[i][1:3]]
                byreg = defaultdict(list)
                for i in mems:
                    _, ra, rb = terms_ph[i]
                    o = rb if ra == h else ra
                    if o != h and o[0] in ("sg", "pair") and o not in placed:
                        byreg[region_of(o)].append((i, o))
                if byreg:
                    reg, lst = max(byreg.items(), key=lambda t: len(t[1]))
                    if len(lst) >= 3:
                        best = (h, reg, lst)
                        break
                banned.add(h)
            if best is None:
                break
            h, reg, lst = best
            base_idx = len(place[reg])
            for i, o in lst:
                place[reg].append(o)
                placed.add(o)
            star_runs[ph].append(([i for i, o in lst], reg, base_idx, h))
            unused -= set(i for i, o in lst)
        leftover[ph] = sorted(unused)

    # remaining aux keys: sorted placement (good for build-run chaining)
    for k in sorted(k for k in shared if k not in placed):
        place["S"].append(k)
    for ph, reg in ((0, "A"), (1, "B")):
        for k in sorted(k for k in only[ph] if k not in placed):
            place[reg].append(k)

    # ---- slot bases --------------------------------------------------
    slot_of = {}
    slot_ph = {0: {}, 1: {}}
    for s in range(S):
        slot_of[("x1", s)] = s
        slot_of[("x0g", s)] = S + s
    baseS = 2 * S
    for i, k in enumerate(place["S"]):
        slot_of[k] = baseS + i
    ov_base = baseS + len(place["S"])
    for i, k in enumerate(place["A"]):
        slot_ph[0][k] = ov_base + i
    for i, k in enumerate(place["B"]):
        slot_ph[1][k] = ov_base + i
    cur = ov_base + max(len(place["A"]), len(place["B"]))
    sq_base = cur
    for k in sqs:
        slot_of[k] = cur
        cur += 1
    pins = sorted(m for m in monos if len(m[0]) >= 2 and len(mono_phase[m]) > 1)
    pin_slot = {}
    for m in pins:
        pin_slot[m] = cur
        cur += 1
    npool = max(8, min(npool, MAX_ARENA - 4 - cur))
    pool_base = cur
    stage_base = pool_base + npool
    n_slots = stage_base + 4

    def res(ph):
        def f(k):
            if k in slot_of:
                return slot_of[k]
            return slot_ph[ph][k]
        return f

    # ---- build runs (dest-aware merging) -----------------------------
    def op_slots(k):
        if k[0] == "pair":
            a, b = k[1]
            return (a, b)  # x1 slots are the segment ids
        s0, s = k[1]
        return (S + s0, s)

    def build_triples(keys, ph):
        f = res(ph)
        return [(f(k),) + op_slots(k) for k in keys]

    keysA = place["S"] + place["A"]
    runsA = _merge_builds(build_triples(place["S"], 0)
                          + build_triples(place["A"], 0))
    runsB = _merge_builds(build_triples(place["B"], 1))

    def needs_x0g(run):
        d0, dd, k, a0, da, b0, db = run
        for i in range(k):
            for v in (a0 + i * da, b0 + i * db):
                if S <= v < 2 * S:
                    return 1
        return 0

    def max_chunk(run):
        d0, dd, k, a0, da, b0, db = run
        mc = 0
        for i in range(k):
            for v in (a0 + i * da, b0 + i * db):
                if v < S:
                    mc = max(mc, v // 4)
        return mc

    runsA.sort(key=lambda r: (needs_x0g(r), max_chunk(r), r[0]))

    # square runs (ACT): dests contiguous from sq_base, srcs = x1 slot = seg
    sq_runs = []
    i = 0
    while i < len(sqs):
        j = i + 1
        if j < len(sqs):
            ds = sqs[j][1] - sqs[i][1]
            while j < len(sqs) and sqs[j][1] - sqs[j - 1][1] == ds:
                j += 1
        sq_runs.append((sq_base + i, j - i, sqs[i][1],
                        (sqs[i + 1][1] - sqs[i][1]) if j - i > 1 else 1))
        i = j

    # ---- terms -------------------------------------------------------
    terms = {0: [], 1: []}  # (mono, A_slot, B_slot)
    d1_mms = {0: [], 1: []}  # (sg_slot, so, c)
    d1B_early = []  # phase-B d1 whose sg is shared (built in phase A)
    for m, uses in sorted(monos.items()):
        segs, s0 = m
        if len(segs) == 1:
            sgk = ("sg", (s0, segs[0]))
            for so, c in uses:
                ph = phase_of_so(so)
                d1_mms[ph].append((res(ph)(sgk), so, c))
                if ph == 1:
                    d1B_early.append(sgk in slot_of)
            continue
    for ph in (0, 1):
        f = res(ph)
        for m, rA, rB in termref[ph]:
            terms[ph].append((m, f(rA), f(rB)))

    region_base = {"S": baseS, "A": ov_base, "B": ov_base}
    term_runs = {0: [], 1: []}
    for ph in (0, 1):
        for members, reg, base_idx, hub in star_runs[ph]:
            a0 = region_base[reg] + base_idx
            term_runs[ph].append((members, a0, 1, res(ph)(hub), 0))
        lo = leftover[ph]
        lruns = _merge_ops([(terms[ph][i][1], terms[ph][i][2]) for i in lo])
        for members, a0, da, b0, db in lruns:
            term_runs[ph].append(([lo[i] for i in members], a0, da, b0, db))

    # readiness of phase-A term runs / d1 mms vs. runsA emission order
    slot_ready = {}
    for ri, (d0, dd, k, *_r) in enumerate(runsA):
        for i in range(k):
            slot_ready[d0 + i * dd] = ri
    nA = len(runsA)

    # terms with no tracked build dep (x0g/sq/x1-only operands) must not sit
    # at the head of the in-order DVE queue: x0g is ready only after the
    # gather matmuls + ACT copies (~10us). Slot them mid-stream instead.
    K_START = min(13, max(0, nA - 1))

    def rdy(slots):
        r = max([slot_ready.get(s, -1) for s in slots] + [-1])
        if r < 0:
            needs_gather = any(S <= s < 2 * S or s >= 2 * S for s in slots)
            return K_START if needs_gather else 0
        return r

    readyA = [min(nA - 1, rdy([s for mi in mem
                               for s in terms[0][mi][1:3]]))
              for (mem, *_r) in term_runs[0]]
    d1_readyA = [min(nA - 1, rdy([sl])) for sl, _, _ in d1_mms[0]]

    # gather emission order: x0g segments by first use in the DVE stream
    first_use = {}
    x1_first = {}
    for ri, run in enumerate(runsA):
        d0, dd, k, a0, da, b0, db = run
        for i in range(k):
            for v in (a0 + i * da, b0 + i * db):
                if S <= v < 2 * S:
                    first_use.setdefault(v - S, ri)
                elif v < S:
                    x1_first.setdefault(v, ri)
    for ph in (0, 1):
        for m, As, Bs in terms[ph]:
            for v in (As, Bs):
                if S <= v < 2 * S:
                    first_use.setdefault(v - S, K_START if ph == 0 else 500)
    gather_order = sorted(range(S), key=lambda s: first_use.get(s, 999))
    x1_order = sorted(range(S), key=lambda s: x1_first.get(s, 999))

    return dict(
        monos=monos, forms=forms, paths=paths, mono_phase=mono_phase,
        runsA=runsA, runsB=runsB, sq_runs=sq_runs,
        terms=terms, term_runs=term_runs, readyA=readyA,
        d1_mms=d1_mms, d1_readyA=d1_readyA,
        pins=pins, pin_slot=pin_slot,
        pool_base=pool_base, npool=npool, stage_base=stage_base,
        n_slots=n_slots, phase_of_so=phase_of_so,
        gather_order=gather_order, x1_order=x1_order,
        d1B_early=d1B_early,
    )


def _schedule(plan):
    """Interleaved emission schedule + full PE matmul order."""
    termsA, termsB = plan["terms"][0], plan["terms"][1]
    runsA_t, runsB_t = plan["term_runs"][0], plan["term_runs"][1]
    monos = plan["monos"]
    phase_of_so = plan["phase_of_so"]
    pin_slot = plan["pin_slot"]
    npool = plan["npool"]
    pool_base = plan["pool_base"]
    nbuildA = len(plan["runsA"])

    runs_after = defaultdict(list)
    for ti, r in enumerate(plan["readyA"]):
        runs_after[r].append(ti)
    d1_after = defaultdict(list)
    for di, r in enumerate(plan["d1_readyA"]):
        d1_after[r].append(di)

    pool_next = [0]

    def alloc_pool(k):
        if pool_next[0] + k > npool:
            pool_next[0] = 0
        lo = pool_next[0]
        pool_next[0] += k
        return pool_base + lo

    term_dest = {}

    def place(ph, terms_ph, run):
        members = run[0]
        dest = alloc_pool(len(members))
        for j, mi in enumerate(members):
            m = terms_ph[mi][0]
            term_dest[(ph, mi)] = (pin_slot[m] if m in pin_slot
                                   else dest + j)

    schedule = []
    for bi in range(-1, nbuildA):
        if bi >= 0:
            schedule.append(("buildA", bi))
        for di in d1_after.get(bi, []):
            schedule.append(("d1A", di))
        for ti in runs_after.get(bi, []):
            place(0, termsA, runsA_t[ti])
            schedule.append(("termA", ti))
            for mi in runsA_t[ti][0]:
                schedule.append(("mmA", mi))
    schedule.append(("drainA",))
    schedule.append(("pinB",))
    # interleave shared-sg d1 matmuls into the B-only build stretch so the
    # PE has real work (and stays at full clock) while DVE rebuilds aux
    nd1B = len(plan["d1_mms"][1])
    early = [di for di in range(nd1B) if plan["d1B_early"][di]]
    late = [di for di in range(nd1B) if not plan["d1B_early"][di]]
    for bi in range(len(plan["runsB"])):
        schedule.append(("buildB", bi))
        if early:
            schedule.append(("d1B", early.pop(0)))
    for di in early + late:
        schedule.append(("d1B", di))
    for ti, run in enumerate(runsB_t):
        place(1, termsB, run)
        schedule.append(("termB", ti))
        for mi in run[0]:
            schedule.append(("mmB", mi))
    schedule.append(("drainB",))

    pin_b_mms = []
    for mi, t in enumerate(termsA):
        m = t[0]
        if m in pin_slot:
            for so, c in monos[m]:
                if phase_of_so(so) == 1:
                    pin_b_mms.append((so, c, pin_slot[m]))

    full_order = []
    for ev in schedule:
        kind = ev[0]
        if kind == "d1A":
            full_order.append(plan["d1_mms"][0][ev[1]])
        elif kind == "d1B":
            full_order.append(plan["d1_mms"][1][ev[1]])
        elif kind == "mmA":
            m = termsA[ev[1]][0]
            dest = term_dest[(0, ev[1])]
            for so, c in monos[m]:
                if phase_of_so(so) == 0:
                    full_order.append((dest, so, c))
        elif kind == "mmB":
            m = termsB[ev[1]][0]
            dest = term_dest[(1, ev[1])]
            for so, c in monos[m]:
                if phase_of_so(so) == 1:
                    full_order.append((dest, so, c))
        elif kind == "pinB":
            for so, c, sl in pin_b_mms:
                full_order.append((sl, so, c))
    # normalize d1 entries (sl, so, c) ordering
    norm = []
    for e in full_order:
        sl, so, c = e
        norm.append((sl, so, c))
    return schedule, term_dest, pin_b_mms, norm


# --------------------------------------------------------------------------
# bass emission
# --------------------------------------------------------------------------

def _build_bass(plan, dtype_name, filler, pool_frac, warmup=16, filler_b=2):
    import concourse.bacc as bacc
    import concourse.mybir as mybir
    from concourse.tile import TileContext

    dt = mybir.dt.float32 if dtype_name == F32 else mybir.dt.bfloat16
    MULT = mybir.AluOpType.mult

    nc = bacc.Bacc("TRN2", debug=False)

    schedule, term_dest, pin_b_mms, full_order = _schedule(plan)
    first_mm, last_mm = {}, {}
    for i, (sl, so, c) in enumerate(full_order):
        if so not in first_mm:
            first_mm[so] = i
        last_mm[so] = i
    n_mms = len(full_order)
    n_slabs = (n_mms + SLAB - 1) // SLAB

    n_slots = plan["n_slots"]
    termsA, termsB = plan["terms"][0], plan["terms"][1]
    runsA_t, runsB_t = plan["term_runs"][0], plan["term_runs"][1]
    monos = plan["monos"]
    phase_of_so = plan["phase_of_so"]
    pin_slot = plan["pin_slot"]

    x1t_d = nc.dram_tensor("x1t", [S * U, ZS], dt, kind="ExternalInput")
    x0_d = nc.dram_tensor("x0w", [NELEM, S * U], dt, kind="ExternalInput")
    oh_d = nc.dram_tensor("oh", [NELEM, ZS], dt, kind="ExternalInput")
    cd_d = nc.dram_tensor("cdiag", [n_slabs * SLAB * U, U], dt,
                          kind="ExternalInput")
    out_d = nc.dram_tensor("outt", [S * U, ZS], dt, kind="ExternalOutput")

    coeff_order = []

    sosA = sorted(set(so for _, so, _ in full_order if so < 8))
    sosB = sorted(set(so for _, so, _ in full_order if so >= 8))
    filler_soA = max(sosA, key=lambda so: last_mm[so]) if sosA else None
    filler_soB = max(sosB, key=lambda so: last_mm[so]) if sosB else None

    with TileContext(nc) as tc:
        with tc.tile_pool(name="persist", bufs=1) as persist, tc.tile_pool(
            name="slab", bufs=3
        ) as slab_pool, tc.tile_pool(name="small", bufs=1) as small:
            arena = persist.tile([U, n_slots * ZS], dt, tag="arena")
            x0_sb = small.tile([NELEM, S * U], dt, tag="x0w")
            oh_sb = small.tile([NELEM, ZS], dt, tag="oh")
            zdiag = small.tile([U, U], dt, tag="zdiag")

            ar = arena[:].rearrange("p (r z) -> p r z", r=n_slots)

            def span(lo, step, k):
                if k == 1:
                    return ar[:, lo, :]
                if step == 0:
                    return ar[:, lo:lo + 1, :].broadcast_to([U, k, ZS])
                return ar[:, lo::step, :][:, 0:k, :]

            def tile2d(slot):
                return ar[:, slot, :]

            nc.gpsimd.memset(zdiag[:], 0.0)
            # x1t chunks lead on the sync queue (first DVE work reads them);
            # oh/x0 (gather inputs) lead on the scalar queue.
            nc.scalar.dma_start(out=oh_sb[:], in_=oh_d[:])
            nc.scalar.dma_start(out=x0_sb[:], in_=x0_d[:])
            # per-segment DMAs in first-use order, alternating the two HW DGE
            # rings: many small transfers run on parallel DMA engines
            for si, s in enumerate(plan.get("x1_order", range(S))):
                q = nc.sync if si % 2 == 0 else nc.scalar
                q.dma_start(
                    out=arena[:, s * ZS:(s + 1) * ZS],
                    in_=x1t_d[s * U:(s + 1) * U, :],
                )

            # squares on ACT (only need x1t)
            for d0, k, s0_, ds in plan["sq_runs"]:
                nc.scalar.activation(
                    span(d0, 1, k),
                    span(s0_, ds, k),
                    mybir.ActivationFunctionType.Square,
                )

            # PE warmup burst: back-to-back junk matmuls off the small oh/x0
            # tensors so the HAM clock gate reaches 2.4 GHz before the
            # gathers + path matmuls.
            with tc.tile_pool(name="gpsum", bufs=4, space="PSUM") as gpsum:
                if warmup > 0:
                    wt = gpsum.tile([U, ZS], mybir.dt.float32, tag="warm",
                                    bufs=1)
                    for i in range(warmup):
                        nc.tensor.matmul(
                            wt[:], x0_sb[:, 0:U], oh_sb[:],
                            start=(i == 0), stop=(i == warmup - 1),
                        )
                for s in plan.get("gather_order", range(S)):
                    pt = gpsum.tile([U, ZS], mybir.dt.float32, tag="gps")
                    nc.tensor.matmul(
                        pt[:], x0_sb[:, s * U:(s + 1) * U], oh_sb[:],
                        start=True, stop=True,
                    )
                    nc.scalar.copy(out=tile2d(S + s), in_=pt[:])

            slab_state = {"tiles": {}, "issued": -1}

            def issue_slab(sj):
                if sj > slab_state["issued"] and sj < n_slabs:
                    st = slab_pool.tile(
                        [U, SLAB * U], dt, tag="slab", name=f"slab{sj}"
                    )
                    slab_state["tiles"][sj] = st
                    slab_state["issued"] = sj
                    q = nc.sync if sj % 2 == 0 else nc.scalar
                    q.dma_start(
                        out=st[:].rearrange("p (d c) -> p d c", d=SLAB),
                        in_=cd_d[sj * SLAB * U:(sj + 1) * SLAB * U, :]
                        .rearrange("(d p) c -> p d c", p=U),
                    )

            def mm(rhs_slot, so, c, acc):
                gi = len(coeff_order)
                coeff_order.append(c)
                sj, sk = gi // SLAB, gi % SLAB
                issue_slab(sj)
                issue_slab(sj + 1)
                issue_slab(sj + 2)
                st = slab_state["tiles"][sj]
                if sk == SLAB - 1:
                    slab_state["tiles"].pop(sj - 3, None)
                nc.tensor.matmul(
                    acc[so][:], st[:, sk * U:(sk + 1) * U], tile2d(rhs_slot),
                    start=(gi == first_mm[so]), stop=(gi == last_mm[so]),
                )

            def emit_build(run, engine):
                d0, dd, k, a0, da, b0, db = run
                engine.tensor_tensor(
                    out=span(d0, dd, k), in0=span(a0, da, k),
                    in1=span(b0, db, k), op=MULT,
                )

            def emit_term_run(run, terms_ph, ph):
                members, a0, da, b0, db = run
                pinned = any(terms_ph[mi][0] in pin_slot for mi in members)
                k = len(members)
                if not pinned and k > 1:
                    d0 = term_dest[(ph, members[0])]
                    nc.vector.tensor_tensor(
                        out=span(d0, 1, k), in0=span(a0, da, k),
                        in1=span(b0, db, k), op=MULT,
                    )
                else:
                    for mi in members:
                        _, As, Bs = terms_ph[mi]
                        nc.vector.tensor_tensor(
                            out=span(term_dest[(ph, mi)], 1, 1),
                            in0=span(As, 1, 1), in1=span(Bs, 1, 1), op=MULT,
                        )

            stage_next = [0]

            def drain(acc, sos):
                for so in sos:
                    sl = plan["stage_base"] + (stage_next[0] % 4)
                    stage_next[0] += 1
                    nc.scalar.copy(out=tile2d(sl), in_=acc[so][:])
                    nc.sync.dma_start(
                        out=out_d[so * U:(so + 1) * U, :], in_=tile2d(sl)
                    )

            n_pool_builds = int(round(len(plan["runsA"]) * pool_frac))
            pool_build_set = set()
            if n_pool_builds:
                # offload evenly-spaced build runs (skip the first few)
                idxs_ = list(range(2, len(plan["runsA"])))
                step = max(1, len(idxs_) // n_pool_builds)
                pool_build_set = set(idxs_[::step][:n_pool_builds])

            with tc.tile_pool(name="accA", bufs=8, space="PSUM") as accpA:
                accA = {so: accpA.tile(
                    [U, ZS], mybir.dt.float32, tag=f"accA{so % 8}",
                    name=f"acc_{so}", bufs=1) for so in sosA}
                for ev in schedule:
                    kind = ev[0]
                    if kind == "buildA":
                        eng = (nc.gpsimd if ev[1] in pool_build_set
                               else nc.vector)
                        emit_build(plan["runsA"][ev[1]], eng)
                        if filler and ev[1] % filler == 0:
                            nc.tensor.matmul(
                                accA[filler_soA][:], zdiag[:], tile2d(0),
                                start=False, stop=False,
                            )
                    elif kind == "d1A":
                        sl, so, c = plan["d1_mms"][0][ev[1]]
                        mm(sl, so, c, accA)
                    elif kind == "termA":
                        emit_term_run(runsA_t[ev[1]], termsA, 0)
                    elif kind == "mmA":
                        mi = ev[1]
                        m = termsA[mi][0]
                        dest = term_dest[(0, mi)]
                        for so, c in monos[m]:
                            if phase_of_so(so) == 0:
                                mm(dest, so, c, accA)
                    elif kind == "drainA":
                        drain(accA, sosA)
                        break
            with tc.tile_pool(name="accB", bufs=8, space="PSUM") as accpB:
                accB = {so: accpB.tile(
                    [U, ZS], mybir.dt.float32, tag=f"accB{so % 8}",
                    name=f"acc_{so}", bufs=1) for so in sosB}
                after = False
                for ev in schedule:
                    kind = ev[0]
                    if kind == "drainA":
                        after = True
                        continue
                    if not after:
                        continue
                    if kind == "pinB":
                        for so, c, sl in pin_b_mms:
                            mm(sl, so, c, accB)
                    elif kind == "buildB":
                        emit_build(plan["runsB"][ev[1]], nc.vector)
                        if filler and ev[1] % filler == 0:
                            nc.tensor.matmul(
                                accB[filler_soB][:], zdiag[:], tile2d(0),
                                start=False, stop=False,
                            )
                    elif kind == "d1B":
                        sl, so, c = plan["d1_mms"][1][ev[1]]
                        mm(sl, so, c, accB)
                    elif kind == "termB":
                        emit_term_run(runsB_t[ev[1]], termsB, 1)
                        if filler_b and ev[1] % filler_b == 0:
                            nc.tensor.matmul(
                                accB[filler_soB][:], zdiag[:], tile2d(0),
                                start=False, stop=False,
                            )
                    elif kind == "mmB":
                        mi = ev[1]
                        m = termsB[mi][0]
                        dest = term_dest[(1, mi)]
                        for so, c in monos[m]:
                            if phase_of_so(so) == 1:
                                mm(dest, so, c, accB)
                    elif kind == "drainB":
                        drain(accB, sosB)

    nc.compile()
    assert len(coeff_order) == n_mms, (len(coeff_order), n_mms)
    return nc, coeff_order


# --------------------------------------------------------------------------
# host wrapper
# --------------------------------------------------------------------------

def kernel(x0, x1, coeff1, coeff2, coeff3, i0, idx1, idx2, idx3):
    global LAST_EXEC_NS, LAST_RESULTS
    from concourse.bass_utils import run_bass_kernel_spmd

    x0 = np.asarray(x0, dtype=np.float32)
    x1 = np.asarray(x1, dtype=np.float32)
    i0 = np.asarray(i0).astype(np.int64)
    idxs = [np.asarray(a) for a in (idx1, idx2, idx3)]
    coeffs = [np.asarray(c, dtype=np.float32) for c in (coeff1, coeff2, coeff3)]

    dtype_name = os.environ.get("KERNEL_DTYPE", "bfloat16")
    npool = int(os.environ.get("KERNEL_NPOOL", "22"))
    filler = int(os.environ.get("KERNEL_FILLER", "1"))
    filler_b = int(os.environ.get("KERNEL_FILLER_B", "2"))
    warmup = int(os.environ.get("KERNEL_WARMUP", "10"))
    pool_frac = float(os.environ.get("KERNEL_POOL_FRAC", "0.0"))
    npdt = np.float32
    if dtype_name != F32:
        import ml_dtypes

        npdt = ml_dtypes.bfloat16

    plan = _plan(idxs, coeffs, npool)
    nc, coeff_order = _build_bass(plan, dtype_name, filler, pool_frac,
                                  warmup=warmup, filler_b=filler_b)
    n_slabs = (len(coeff_order) + SLAB - 1) // SLAB
    cdiag = np.zeros((n_slabs * SLAB * U, U), dtype=npdt)
    for gi, c in enumerate(coeff_order):
        blk = cdiag[gi * U:(gi + 1) * U, :]
        np.fill_diagonal(blk, np.asarray(c, dtype=npdt))

    in_maps = []
    eye = np.arange(NELEM)
    x0c = x0.astype(npdt)
    for c in range(NCORES):
        zl, zh = c * ZS, (c + 1) * ZS
        shard = x1[zl:zh]
        x1t = np.ascontiguousarray(
            shard.reshape(ZS, S, U).transpose(1, 2, 0).reshape(S * U, ZS)
        ).astype(npdt)
        oh = (i0[zl:zh][None, :] == eye[:, None]).astype(npdt)
        in_maps.append({"x1t": x1t, "x0w": x0c, "oh": oh, "cdiag": cdiag})

    trace = os.environ.get("BASS_TRACE", "") not in ("", "0")
    trace_cores = None
    tc_env = os.environ.get("KERNEL_TRACE_CORES", "")
    if tc_env:
        trace_cores = [int(x) for x in tc_env.split(",")]
    res = run_bass_kernel_spmd(
        nc, in_maps, core_ids=list(range(NCORES)), trace=trace,
        trace_cores=trace_cores,
    )
    LAST_EXEC_NS = res.exec_time_ns
    LAST_RESULTS = res

    have_so = set(so for _, _, _, so, _ in plan["paths"])
    out = np.empty((Z, S * U), dtype=np.float32)
    for c in range(NCORES):
        outt = np.asarray(res.results[c]["outt"], dtype=np.float32)
        o = outt.reshape(S, U, ZS).transpose(2, 0, 1).copy()
        for so in range(S):
            if so not in have_so:
                o[:, so, :] = 0.0
        out[c * ZS:(c + 1) * ZS] = o.reshape(ZS, S * U)
    return out
